# revision 1
# baseline (speedup 1.0000x reference)
"""NT-Xent loss kernel, v11 (47.86us cost-model, rel err 1.6e-3).

vs v2 (64.1us -> 52.1us cost-model, rel err 3.4e-3 -> 1.9e-4):
- host-side rotation makes every core's q-block rows 0-511, so zq /
  distq are SBUF views and zqT is a column view of zT (no extra DMAs,
  transposes, or copies for the q side);
- f32r dtype end to end for the z data path: sim/W matmuls at
  1 cycle/row (vs 4 for fp32) with near-fp32 precision, transposes at
  1.5 cycles/row, no bf16 conversion copies;
- one manual InstLoadActFuncSet of the ln+exp table set at program
  start; Ln and Exp then never reload tables (v2 thrashed 8x1283ns);
- norms: per-chunk square+reduce on DVE; nn=sqrt(norm2) and
  rn=rsqrt(norm2) via ln/exp on ACT (high_priority so the scheduler
  slots them between the big exp activations); z scaled in place per
  chunk: first tile x*=rn on DVE (two tiles for the last chunk),
  rest /=nn on GpSimd normalize_recip (splitting the scale across
  engines un-gates the per-chunk transpose chain, -2.5us);
- W = dist^T z_hat directly (32 f32r matmuls, no dist scaling),
  issued under tc.high_priority() so the scheduler slots them into PE
  gaps as soon as the last scaled z chunk lands (-1.6us tail);
- 8-chunk z DMA pipeline; chunk-pair transposes into [128,1024] PSUM
  tiles, copies split DVE (d=0) / ACT (d=1); single output DMA.

Known HW landmines (probed): tensor_tensor_reduce crashes the exec
unit; gpsimd tensor_scalar on f32r or int dtypes crashes; the
verifier requires f32r matmul inputs to be produced typed f32r
(normalize_recip and DVE/ACT copies qualify, bitcast writes do not);
transpose-mode matmul requires a permutation-matrix rhs.
"""

import numpy as np
from contextlib import ExitStack

N = 4096
D = 256
C = 100
B = 2048
N_CORES = 8
RPC = 512
NT = 32           # z row-tiles of 128
NCH = 8           # z DMA chunks (4 tiles each)
E_CONST = float(np.e)
MAGIC = 0x5F3759DF

_PROG = None


def _build_program():
    import concourse.bass as bass
    import concourse.tile as tile
    from concourse import bacc, mybir, masks

    f32 = mybir.dt.float32
    f32r = mybir.dt.float32r
    i32 = mybir.dt.int32
    MULT = mybir.AluOpType.mult
    ADD = mybir.AluOpType.add
    XOR = mybir.AluOpType.bitwise_xor
    SHR = mybir.AluOpType.arith_shift_right
    EXP = mybir.ActivationFunctionType.Exp
    LN = mybir.ActivationFunctionType.Ln
    AX = mybir.AxisListType.X

    nc = bacc.Bacc(
        "TRN2",
        target_bir_lowering=False,
        debug=False,
        enable_asserts=False,
        num_devices=N_CORES,
    )

    z = nc.dram_tensor("z", [N, D], f32r, kind="ExternalInput").ap()
    dist = nc.dram_tensor("dist", [B, C], f32r, kind="ExternalInput").ap()
    out = nc.dram_tensor("out", [128, 8], f32, kind="ExternalOutput").ap()

    with tile.TileContext(nc) as tc, ExitStack() as ctx:
        per = ctx.enter_context(tc.tile_pool(name="persist", bufs=1))

        # preload the ln+exp table set once; the insertion pass then adds
        # no further table loads for Ln or Exp anywhere in the program.
        nc.scalar.add_instruction(mybir.InstLoadActFuncSet(
            name=f"I-{nc.next_id()}", ins=[], outs=[], act_func_set_id=6))

        ident = per.tile([128, 128], f32)
        masks.make_identity(nc, ident[:])
        ident_r = per.tile([128, 128], f32r)
        nc.vector.tensor_copy(out=ident_r[:], in_=ident[:])

        zn = per.tile([128, NT * 256], f32r)
        zT = per.tile([128, 2 * 4096], f32r)
        dist_sb = per.tile([128, 16 * C], f32r)
        norm2 = per.tile([128, NT], f32)
        lnn = per.tile([128, NT], f32)
        nn = per.tile([128, NT], f32)
        rn = per.tile([128, NT], f32)
        wt_sb = per.tile([128, 256], f32)
        w_sb = per.tile([128, 2 * C], f32r)
        q_sb = per.tile([128, C], f32)
        junk = per.tile([128, C], f32)
        S_parts = per.tile([128, 16], f32)
        out_sb = per.tile([128, 8], f32)
        exp_scr = per.tile([128, 1024], f32)

        # ---------------- DMAs: 8 z chunks (4 tiles each), then dist --------
        for ch in range(NCH):
            nc.sync.dma_start(
                out=zn[:, ch * 1024:(ch + 1) * 1024].rearrange(
                    "p (t j) -> p t j", j=256),
                in_=z[ch * 512:(ch + 1) * 512, :].rearrange(
                    "(t p) j -> p t j", p=128),
            )
        nc.sync.dma_start(
            out=dist_sb[:].rearrange("p (t c) -> p t c", c=C),
            in_=dist.rearrange("(t p) c -> p t c", p=128),
        )

        # ------- norms: squares (DVE early / Pool late), reduce on DVE ----
        sqp = ctx.enter_context(tc.tile_pool(name="sqp", bufs=2))

        def norms_chunk(ch):
            sq = sqp.tile([128, 1024], f32, tag="sq")
            src_v = zn[:, ch * 1024:(ch + 1) * 1024].bitcast(f32)
            nc.vector.tensor_tensor(out=sq[:], in0=src_v, in1=src_v, op=MULT)
            nc.vector.tensor_reduce(
                out=norm2[:, ch * 4:(ch + 1) * 4],
                in_=sq[:].rearrange("p (t j) -> p t j", j=256),
                axis=AX, op=ADD,
            )

        def sqrt_chunk(ch):
            # nn = sqrt(norm2), rn = rsqrt(norm2) via ln/exp (set-6 tables)
            sl = slice(ch * 4, (ch + 1) * 4)
            with tc.high_priority():
                nc.scalar.activation(lnn[:, sl], norm2[:, sl], LN)
                nc.scalar.activation(nn[:, sl], lnn[:, sl], EXP, scale=0.5)
                nc.scalar.activation(rn[:, sl], lnn[:, sl], EXP, scale=-0.5)

        with tc.tile_pool(name="psum_tr", bufs=2, space="PSUM") as ptr, \
                tc.tile_pool(name="psum_c", bufs=2, space="PSUM") as pc:

            def transpose_pair(ch0):
                # 2 ptr tiles per chunk-pair: same-d for 8 z-tiles, so the
                # PSUM->SBUF copy is one [128,1024] op; d=0 on DVE, d=1 ACT.
                for d in range(2):
                    pt = ptr.tile([128, 1024], f32r, tag="tr")
                    for k in range(8):
                        t = ch0 * 4 + k
                        nc.tensor.transpose(
                            pt[:, k * 128:(k + 1) * 128],
                            zn[:, t * 256 + d * 128: t * 256 + d * 128 + 128],
                            ident_r[:],
                        )
                    dst = zT[:, d * 4096 + ch0 * 512: d * 4096 + (ch0 + 2) * 512]
                    if d == 0:
                        nc.vector.tensor_copy(out=dst, in_=pt[:])
                    else:
                        nc.scalar.copy(out=dst, in_=pt[:])

            def scale_chunk(ch):
                # z -> z_hat in place; split DVE (x*rn) / Pool (x/nn)
                for t in range(ch * 4, ch * 4 + 4):
                    if t % 4 == 0 or (ch == 7 and t % 4 == 1):
                        nc.vector.tensor_scalar(
                            out=zn[:, t * 256:(t + 1) * 256],
                            in0=zn[:, t * 256:(t + 1) * 256],
                            scalar1=rn[:, t:t + 1],
                            scalar2=None, op0=MULT)
                    else:
                        nc.gpsimd.normalize_recip(
                            out_ap=zn[:, t * 256:(t + 1) * 256],
                            in_ap=zn[:, t * 256:(t + 1) * 256].bitcast(f32),
                            denom_ap=nn[:, t:t + 1])

            def sims_group(j):
                # sim rows 0-511 x cols [j*1024,(j+1)*1024), exp row-sums
                for rt in range(4):
                    ps = pc.tile([128, 1024], f32, tag="sim")
                    for cc in range(2):
                        col0 = j * 1024 + cc * 512
                        for d in range(2):
                            nc.tensor.matmul(
                                ps[:, cc * 512:(cc + 1) * 512],
                                lhsT=zT[:, d * 4096 + rt * 128:
                                        d * 4096 + (rt + 1) * 128],
                                rhs=zT[:, d * 4096 + col0:
                                       d * 4096 + col0 + 512],
                                start=(d == 0), stop=(d == 1),
                            )
                    nc.scalar.activation(
                        exp_scr[:], ps[:], EXP,
                        accum_out=S_parts[:, rt * 4 + j: rt * 4 + j + 1],
                    )

            # chunk pipeline: norms -> rsqrt -> scale -> transposes,
            # sims for col-group j after chunks 2j, 2j+1 are transposed.
            # tile_set_cur_wait: floor each chunk's stage at its real DMA
            # arrival so the scheduler doesn't front-load late-chunk work
            # (its internal DMA model is optimistic) and head-of-line-block
            # the in-order engine queues.
            for ch in range(NCH):
                tc.tile_set_cur_wait((2.0 + 1.6 * ch) / 1000.0)
                norms_chunk(ch)
                sqrt_chunk(ch)
                scale_chunk(ch)
                if ch % 2 == 1:
                    transpose_pair(ch - 1)
                    sims_group(ch // 2)

        # ---------------- nominator: W = (rn*dist)^T z, P = zq_hat W -------
        with tc.tile_pool(name="psum_b", bufs=2, space="PSUM") as pb:
            wt_ps = pb.tile([128, 256], f32, tag="wt")
            with tc.high_priority():
                for t in range(NT):
                    nc.tensor.matmul(
                        wt_ps[0:C, :],
                        lhsT=dist_sb[:, (t % 16) * C:((t % 16) + 1) * C],
                        rhs=zn[:, t * 256:(t + 1) * 256],
                        start=(t == 0), stop=(t == NT - 1),
                    )
            with tc.high_priority():
                nc.vector.tensor_copy(out=wt_sb[0:C, :], in_=wt_ps[0:C, :])
                for d in range(2):
                    w_ps = pb.tile([128, 128], f32, tag="wq")
                    nc.tensor.transpose(
                        w_ps[:, 0:C],
                        wt_sb[0:C, d * 128:(d + 1) * 128],
                        ident[0:C, 0:C],
                    )
                    nc.vector.tensor_copy(out=w_sb[:, d * C:(d + 1) * C],
                                          in_=w_ps[:, 0:C])
            for rt in range(4):
                q_ps = pb.tile([128, 128], f32, tag="wq")
                for d in range(2):
                    nc.tensor.matmul(
                        q_ps[:, 0:C],
                        lhsT=zT[:, d * 4096 + rt * 128:
                                d * 4096 + (rt + 1) * 128],
                        rhs=w_sb[:, d * C:(d + 1) * C],
                        start=(d == 0), stop=(d == 1),
                    )
                nc.vector.tensor_copy(out=q_sb[:], in_=q_ps[:, 0:C])
                nc.vector.tensor_mul(junk[:], q_sb[:],
                                     dist_sb[:, rt * C:(rt + 1) * C].bitcast(f32))
                nc.vector.tensor_reduce(out=out_sb[:, 4 + rt:5 + rt],
                                        in_=junk[:], axis=AX, op=ADD)
            nc.vector.tensor_scalar(out=out_sb[:, 4:8], in0=out_sb[:, 4:8],
                                    scalar1=-1.0, scalar2=None, op0=ADD)

        for rt in range(4):
            nc.vector.tensor_reduce(
                out=out_sb[:, rt:rt + 1], in_=S_parts[:, rt * 4:rt * 4 + 4],
                axis=AX, op=ADD,
            )
        nc.vector.tensor_scalar(out=out_sb[:, 0:4], in0=out_sb[:, 0:4],
                                scalar1=-E_CONST, scalar2=None, op0=ADD)

        nc.sync.dma_start(out=out[:], in_=out_sb[:])

    nc.finalize()
    return nc


def _get_program():
    global _PROG
    if _PROG is None:
        _PROG = _build_program()
    return _PROG


def kernel(z_i, z_j, z_n, dist_labels):
    from concourse.bass_utils import run_bass_kernel_spmd

    nc = _get_program()

    z_full = np.ascontiguousarray(
        np.concatenate([z_i, z_j], axis=0), dtype=np.float32
    )
    dist = np.ascontiguousarray(dist_labels, dtype=np.float32)

    in_maps = []
    for c in range(N_CORES):
        r0 = c * RPC
        in_maps.append({
            "z": np.ascontiguousarray(np.roll(z_full, -r0, axis=0)),
            "dist": np.ascontiguousarray(np.roll(dist, -r0, axis=0)),
        })

    res = run_bass_kernel_spmd(nc, in_maps, list(range(N_CORES))).results

    S = np.empty(N, np.float64)
    P = np.empty(N, np.float64)
    for c in range(N_CORES):
        o = res[c]["out"]
        S[c * RPC:(c + 1) * RPC] = o[:, 0:4].T.reshape(RPC).astype(np.float64)
        P[c * RPC:(c + 1) * RPC] = o[:, 4:8].T.reshape(RPC).astype(np.float64)

    return np.float32((P / S).sum() / N)



# revision 11
# speedup vs baseline: 1.0993x; 1.0993x over previous
"""NT-Xent loss kernel, v15: symmetric blocks, engine-legal assignment.

Decomposition (vs v11 baseline at 47.9us): after the host rotation each
core computes sim blocks (q rows 0-511) x (local cols 0..2559) = B0..B4
only. Row sums come from exp accum_out; the missing col groups m=5,6,7
are COLUMN sums of blocks B3,B2,B1 on cores i+5,i+6,i+7 (sim symmetry),
taken with ones-vector matmuls over retained exp values and combined on
the host. B4 is computed by both ends of its pair (row sums only).

Engine legality learned the hard way:
- GPSIMD supports only its custom ISA ops (normalize_recip etc), no
  generic tensor_scalar / scalar_tensor_tensor, and cannot touch PSUM.
- Activation table set 6 = {ln, exp, square, copy, ...}; Rsqrt banned.

Assignment:
- norms: scalar_tensor_tensor (x*1)*x + accum_out, all on DVE.
- rn = exp(-0.5 ln norm2) on ACT (solo chains ch0-4, one batch ch5-7);
  nn = norm2*rn on DVE (feeds normalize_recip).
- scale rows 0..2559: ch0-2 all Pool normalize_recip; ch3-4 k0,k1 via
  one DVE broadcast STT + k2,k3 Pool (keeps Pool's serial queue short
  near the tail). Rows 2560+: rn folded into dist via Pool
  normalize_recip (sdist), W uses raw z there.
- transpose copies: d0 DVE, d1 ACT.
- sims in PSUM granules A=[128,1536] (b0-b2) / B=[128,1024] (b3,b4),
  one exp+accum each; granule pools decouple PE from ACT.
- W matmuls interleaved into the chunk loop; P path fused via STT.
- colsums accumulate in a 3-bank PSUM window after the W pool closes.
"""

import numpy as np
from contextlib import ExitStack

N = 4096
D = 256
C = 100
B = 2048
N_CORES = 8
RPC = 512
NT = 32
NCH = 8
NBLK = 5
SIMW = NBLK * 512
NSC = NBLK * 4
E_CONST = float(np.e)

_PROG = None


def _build_program():
    import concourse.bass as bass
    import concourse.tile as tile
    from concourse import bacc, mybir, masks

    f32 = mybir.dt.float32
    f32r = mybir.dt.float32r
    MULT = mybir.AluOpType.mult
    EXP = mybir.ActivationFunctionType.Exp
    LN = mybir.ActivationFunctionType.Ln

    nc = bacc.Bacc(
        "TRN2",
        target_bir_lowering=False,
        debug=False,
        enable_asserts=False,
        num_devices=N_CORES,
    )

    z = nc.dram_tensor("z", [N, D], f32r, kind="ExternalInput").ap()
    dist = nc.dram_tensor("dist", [B, C], f32r, kind="ExternalInput").ap()
    out = nc.dram_tensor("out", [128, 12], f32, kind="ExternalOutput").ap()
    out2 = nc.dram_tensor("out2", [1, 1536], f32, kind="ExternalOutput").ap()

    with tile.TileContext(nc) as tc, ExitStack() as ctx:
        per = ctx.enter_context(tc.tile_pool(name="persist", bufs=1))

        nc.scalar.add_instruction(mybir.InstLoadActFuncSet(
            name=f"I-{nc.next_id()}", ins=[], outs=[], act_func_set_id=6))

        ident = per.tile([128, 128], f32)
        masks.make_identity(nc, ident[:])
        ident_r = per.tile([128, 128], f32r)
        nc.vector.tensor_copy(out=ident_r[:], in_=ident[:])
        ones_f = per.tile([128, 1], f32)
        nc.vector.memset(ones_f[:], 1.0)
        ones_r = per.tile([128, 1], f32r)
        nc.vector.tensor_copy(out=ones_r[:], in_=ones_f[:])

        zn = per.tile([128, NT * 256], f32r)
        zT = per.tile([128, 2 * SIMW], f32r)
        exp_sb = per.tile([128, 4 * SIMW], f32r)
        dist_sb = per.tile([128, 4 * 400], f32r)
        sdist = per.tile([128, 12 * 100], f32r)
        norm2 = per.tile([128, NT], f32)
        lnn = per.tile([128, NT], f32)
        nn = per.tile([128, NT], f32)
        rn = per.tile([128, NT], f32)
        wt_sb = per.tile([128, 256], f32)
        w_sb = per.tile([128, 2 * 256], f32r)
        junk = per.tile([128, C], f32)
        out_sb = per.tile([128, 12], f32)
        cs_sb = per.tile([1, 1536], f32)
        nc.vector.memset(w_sb[:].bitcast(f32), 0.0)

        # ---------------- DMAs: 8 z chunks, then dist ----------------------
        for ch in range(NCH):
            nc.sync.dma_start(
                out=zn[:, ch * 1024:(ch + 1) * 1024].rearrange(
                    "p (q j) -> p q j", j=256),
                in_=z[ch * 512:(ch + 1) * 512, :].rearrange(
                    "(p q) j -> p q j", q=4),
            )
        nc.sync.dma_start(
            out=dist_sb[:].rearrange("p (ch q c) -> p ch q c", q=4, c=C),
            in_=dist.rearrange("(ch p q) c -> p ch q c", p=128, q=4),
        )

        # ---------------- PE warm-up: ramp tensor clock --------------------
        with tc.tile_pool(name="warm", bufs=1, space="PSUM") as wp:
            warm = wp.tile([128, 128], f32, tag="warm")
            for _ in range(16):
                nc.tensor.matmul(warm[:], lhsT=ident_r[:], rhs=ident_r[:],
                                 start=True, stop=True)

        sqd = ctx.enter_context(tc.tile_pool(name="sqd", bufs=3))

        def arrival(ch):
            return 2.9 + 1.46 * ch

        def norms_chunk(ch):
            with tc.high_priority():
                for k in range(4):
                    t = 4 * ch + k
                    src = zn[:, t * 256:(t + 1) * 256].bitcast(f32)
                    s = sqd.tile([128, 256], f32, tag="sq", name=f"sq{t}")
                    nc.vector.scalar_tensor_tensor(
                        out=s[:], in0=src, scalar=1.0, in1=src,
                        op0=MULT, op1=MULT,
                        accum_out=norm2[:, t:t + 1])

        def rn_chain(t0, nt, need_nn):
            sl = slice(t0, t0 + nt)
            with tc.high_priority():
                nc.scalar.activation(lnn[:, sl], norm2[:, sl], LN)
                nc.scalar.activation(rn[:, sl], lnn[:, sl], EXP, scale=-0.5)
                if need_nn:
                    nc.vector.tensor_tensor(out=nn[:, sl], in0=norm2[:, sl],
                                            in1=rn[:, sl], op=MULT)

        def scale_chunk(ch):
            t0 = 4 * ch
            with tc.high_priority():
                if ch >= 3:
                    dst = zn[:, t0 * 256:(t0 + 2) * 256]
                    bc = rn[:, t0:t0 + 2].unsqueeze(2).to_broadcast(
                        [128, 2, 256])
                    nc.vector.scalar_tensor_tensor(
                        out=dst.rearrange("p (k j) -> p k j", j=256),
                        in0=dst.bitcast(f32).rearrange(
                            "p (k j) -> p k j", j=256),
                        scalar=1.0, in1=bc, op0=MULT, op1=MULT)
                    ks = (2, 3)
                else:
                    ks = (0, 1, 2, 3)
                for k in ks:
                    t = 4 * ch + k
                    d2 = zn[:, t * 256:(t + 1) * 256]
                    nc.gpsimd.normalize_recip(
                        out_ap=d2, in_ap=d2.bitcast(f32),
                        denom_ap=nn[:, t:t + 1])

        def transpose_chunk(ptp, ch):
            for d in range(2):
                pt = ptp.tile([128, 512], f32r, tag="tr", name=f"pt{ch}_{d}")
                with tc.high_priority():
                    for k in range(4):
                        t = 4 * ch + k
                        nc.tensor.transpose(
                            pt[:, k * 128:(k + 1) * 128],
                            zn[:, t * 256 + d * 128: t * 256 + d * 128 + 128],
                            ident_r[:],
                        )
                    dst = zT[:, d * SIMW + ch * 512: d * SIMW + (ch + 1) * 512]
                    if d == 0:
                        nc.vector.tensor_copy(out=dst, in_=pt[:])
                    else:
                        nc.scalar.copy(out=dst, in_=pt[:])

        def w_tiles(ts):
            for t in ts:
                if t < NSC:
                    lhsT = dist_sb[:, (t // 4 % 4) * 400 + (t % 4) * 100:
                                   (t // 4 % 4) * 400 + (t % 4) * 100 + 100]
                else:
                    lhsT = sdist[:, (t - NSC) * 100:(t - NSC + 1) * 100]
                nc.tensor.matmul(
                    wt_ps[0:C, :], lhsT=lhsT,
                    rhs=zn[:, t * 256:(t + 1) * 256],
                    start=(t == 0), stop=(t == NT - 1),
                )

        gA = ctx.enter_context(tc.tile_pool(name="gA", bufs=1, space="PSUM"))
        gB = ctx.enter_context(tc.tile_pool(name="gB", bufs=1, space="PSUM"))
        pw_cm = tc.tile_pool(name="pw", bufs=1, space="PSUM")
        pw = pw_cm.__enter__()
        wt_ps = pw.tile([128, 256], f32, tag="wt", name="wt_ps")

        def sims_into(g, rt, b, col0):
            for d in range(2):
                nc.tensor.matmul(
                    g[:, col0:col0 + 512],
                    lhsT=zT[:, d * SIMW + rt * 128: d * SIMW + (rt + 1) * 128],
                    rhs=zT[:, d * SIMW + b * 512: d * SIMW + (b + 1) * 512],
                    start=(d == 0), stop=(d == 1),
                )

        def exp_granule(g, rt, gi, width):
            nc.scalar.activation(
                out=exp_sb[:, rt * SIMW + gi * 1536:
                           rt * SIMW + gi * 1536 + width],
                in_=g[:, 0:width], func=EXP,
                accum_out=out_sb[:, gi * 4 + rt: gi * 4 + rt + 1])

        def granules_for(rt):
            a = gA.tile([128, 1536], f32, tag="gA", name=f"g{rt}a")
            for b in range(3):
                sims_into(a, rt, b, b * 512)
            exp_granule(a, rt, 0, 1536)
            bb = gB.tile([128, 1024], f32, tag="gB", name=f"g{rt}b")
            sims_into(bb, rt, 3, 0)
            sims_into(bb, rt, 4, 512)
            exp_granule(bb, rt, 1, 1024)

        # ---------------- chunks 0-4 + strip rt0 granules -------------------
        with tc.tile_pool(name="pt", bufs=2, space="PSUM") as ptp:
            a0 = b0 = None
            for ch in range(NBLK):
                tc.tile_set_cur_wait(arrival(ch) / 1000.0)
                norms_chunk(ch)
                tc.tile_set_cur_wait((arrival(ch) + 0.40) / 1000.0)
                rn_chain(4 * ch, 4, need_nn=True)
                tc.tile_set_cur_wait((arrival(ch) + 0.75) / 1000.0)
                scale_chunk(ch)
                tc.tile_set_cur_wait((arrival(ch) + 1.30) / 1000.0)
                transpose_chunk(ptp, ch)
                tc.tile_set_cur_wait((arrival(ch) + 1.10) / 1000.0)
                w_tiles(range(4 * ch, 4 * ch + 4))
                tc.tile_set_cur_wait((arrival(ch) + 1.95) / 1000.0)
                if ch == 0:
                    a0 = gA.tile([128, 1536], f32, tag="gA", name="g0a")
                    sims_into(a0, 0, 0, 0)
                elif ch == 1:
                    sims_into(a0, 0, 1, 512)
                elif ch == 2:
                    sims_into(a0, 0, 2, 1024)
                    exp_granule(a0, 0, 0, 1536)
                elif ch == 3:
                    b0 = gB.tile([128, 1024], f32, tag="gB", name="g0b")
                    sims_into(b0, 0, 3, 0)
                else:
                    sims_into(b0, 0, 4, 512)
                    exp_granule(b0, 0, 1, 1024)

        for ch in range(NBLK, NCH):
            tc.tile_set_cur_wait(arrival(ch) / 1000.0)
            norms_chunk(ch)
        tc.tile_set_cur_wait((arrival(7) + 0.40) / 1000.0)
        rn_chain(NSC, 12, need_nn=True)
        with tc.high_priority():
            for t in range(NSC, NT):
                src = dist_sb[:, (t // 4 % 4) * 400 + (t % 4) * 100:
                              (t // 4 % 4) * 400 + (t % 4) * 100 + 100]
                nc.gpsimd.normalize_recip(
                    out_ap=sdist[:, (t - NSC) * 100:(t - NSC + 1) * 100],
                    in_ap=src.bitcast(f32),
                    denom_ap=nn[:, t:t + 1])

        # ---------------- strips 1-3, W tail, P path ------------------------
        granules_for(1)
        tc.tile_set_cur_wait((arrival(7) + 1.5) / 1000.0)
        w_tiles(range(NSC, NT))
        granules_for(2)

        nc.vector.tensor_copy(out=wt_sb[0:C, :], in_=wt_ps[0:C, :])
        for d in range(2):
            w_ps = pw.tile([128, 128], f32, tag="wt", name=f"w_ps{d}")
            nc.tensor.transpose(
                w_ps[:, 0:C],
                wt_sb[0:C, d * 128:(d + 1) * 128],
                ident[0:C, 0:C],
            )
            nc.vector.tensor_copy(out=w_sb[:, d * 256:d * 256 + C],
                                  in_=w_ps[:, 0:C])

        granules_for(3)

        for rt in range(4):
            q_ps = pw.tile([128, 256], f32, tag="wt", name=f"q_ps{rt}")
            for d in range(2):
                nc.tensor.matmul(
                    q_ps[:], lhsT=zT[:, d * SIMW + rt * 128:
                                     d * SIMW + (rt + 1) * 128],
                    rhs=w_sb[:, d * 256:(d + 1) * 256],
                    start=(d == 0), stop=(d == 1),
                )
            nc.vector.scalar_tensor_tensor(
                out=junk[:], in0=q_ps[:, 0:C], scalar=1.0,
                in1=dist_sb[:, rt * 100:rt * 100 + 100].bitcast(f32),
                op0=MULT, op1=MULT,
                accum_out=out_sb[:, 8 + rt:9 + rt])
        pw_cm.__exit__(None, None, None)

        # ---------------- column sums of blocks B1-B3 -----------------------
        with tc.tile_pool(name="cs", bufs=1, space="PSUM") as csp:
            cs_ps = csp.tile([128, 1536], f32, tag="cs")
            for rt in range(4):
                for k in (1, 2, 3):
                    nc.tensor.matmul(
                        cs_ps[0:1, (k - 1) * 512:k * 512], lhsT=ones_r[:],
                        rhs=exp_sb[:, rt * SIMW + k * 512:
                                   rt * SIMW + (k + 1) * 512],
                        start=(rt == 0), stop=(rt == 3),
                    )
            nc.scalar.copy(out=cs_sb[:, 0:512], in_=cs_ps[0:1, 0:512])
            nc.vector.tensor_copy(out=cs_sb[:, 512:1024],
                                  in_=cs_ps[0:1, 512:1024])
            nc.scalar.copy(out=cs_sb[:, 1024:1536],
                           in_=cs_ps[0:1, 1024:1536])
            nc.sync.dma_start(out=out2[:], in_=cs_sb[:])

        nc.sync.dma_start(out=out[:], in_=out_sb[:])

    nc.finalize()
    return nc


def _get_program():
    global _PROG
    if _PROG is None:
        _PROG = _build_program()
    return _PROG


def kernel(z_i, z_j, z_n, dist_labels):
    from concourse.bass_utils import run_bass_kernel_spmd

    nc = _get_program()

    z_full = np.ascontiguousarray(
        np.concatenate([z_i, z_j], axis=0), dtype=np.float32
    )
    dist = np.ascontiguousarray(dist_labels, dtype=np.float32)

    in_maps = []
    for c in range(N_CORES):
        r0 = c * RPC
        in_maps.append({
            "z": np.ascontiguousarray(np.roll(z_full, -r0, axis=0)),
            "dist": np.ascontiguousarray(np.roll(dist, -r0, axis=0)),
        })

    res = run_bass_kernel_spmd(nc, in_maps, list(range(N_CORES))).results

    S = np.zeros(N, np.float64)
    P = np.empty(N, np.float64)
    idx = np.arange(RPC)
    for c in range(N_CORES):
        o = res[c]["out"].astype(np.float64)
        g = (idx + RPC * c) % N
        # device order p*4+rt == local row 4p+rt
        S[g] += (o[:, 0:4] + o[:, 4:8]).reshape(RPC)
        P[g] = o[:, 8:12].reshape(RPC)
        cs = res[c]["out2"].astype(np.float64).reshape(3, 4, 128)
        for k in (1, 2, 3):
            # block col j=kk*128+p -> local row 512k + 4p + kk
            cs_r = cs[k - 1].T.reshape(RPC)
            gk = (idx + RPC * (c + k)) % N
            S[gk] += cs_r

    S -= E_CONST
    P -= 1.0
    return np.float32((P / S).sum() / N)


# revision 23
# speedup vs baseline: 1.2204x; 1.1102x over previous
"""NT-Xent loss kernel, v15: symmetric blocks, engine-legal assignment.

Decomposition (vs v11 baseline at 47.9us): after the host rotation each
core computes sim blocks (q rows 0-511) x (local cols 0..2559) = B0..B4
only. Row sums come from exp accum_out; the missing col groups m=5,6,7
are COLUMN sums of blocks B3,B2,B1 on cores i+5,i+6,i+7 (sim symmetry),
taken with ones-vector matmuls over retained exp values and combined on
the host. B4 is computed by both ends of its pair (row sums only).

Engine legality learned the hard way:
- GPSIMD supports only its custom ISA ops (normalize_recip etc), no
  generic tensor_scalar / scalar_tensor_tensor, and cannot touch PSUM.
- Activation table set 6 = {ln, exp, square, copy, ...}; Rsqrt banned.

Assignment:
- norms: scalar_tensor_tensor (x*1)*x + accum_out, all on DVE.
- rn = exp(-0.5 ln norm2) on ACT (solo chains ch0-4, one batch ch5-7);
  nn = norm2*rn on DVE (feeds normalize_recip).
- scale rows 0..2559: ch0-2 all Pool normalize_recip; ch3-4 k0,k1 via
  one DVE broadcast STT + k2,k3 Pool (keeps Pool's serial queue short
  near the tail). Rows 2560+: rn folded into dist via Pool
  normalize_recip (sdist), W uses raw z there.
- transpose copies: d0 DVE, d1 ACT.
- sims in PSUM granules A=[128,1536] (b0-b2) / B=[128,1024] (b3,b4),
  one exp+accum each; granule pools decouple PE from ACT.
- W matmuls interleaved into the chunk loop; P path fused via STT.
- colsums accumulate in a 3-bank PSUM window after the W pool closes.
"""

import numpy as np
from contextlib import ExitStack

N = 4096
D = 256
C = 100
B = 2048
N_CORES = 8
RPC = 512
NT = 32
NCH = 8
NBLK = 5
SIMW = NBLK * 512
NSC = NBLK * 4
E_CONST = float(np.e)

_PROG = None


def _build_program():
    import concourse.bass as bass
    import concourse.tile as tile
    from concourse import bacc, mybir, masks

    f32 = mybir.dt.float32
    f32r = mybir.dt.float32r
    MULT = mybir.AluOpType.mult
    EXP = mybir.ActivationFunctionType.Exp
    LN = mybir.ActivationFunctionType.Ln

    nc = bacc.Bacc(
        "TRN2",
        target_bir_lowering=False,
        debug=False,
        enable_asserts=False,
        num_devices=N_CORES,
    )

    z = nc.dram_tensor("z", [N, D], f32r, kind="ExternalInput").ap()
    dist = nc.dram_tensor("dist", [B, C], f32r, kind="ExternalInput").ap()
    out = nc.dram_tensor("out", [128, 12], f32, kind="ExternalOutput").ap()
    out2 = nc.dram_tensor("out2", [1, 1536], f32, kind="ExternalOutput").ap()

    with tile.TileContext(nc) as tc, ExitStack() as ctx:
        per = ctx.enter_context(tc.tile_pool(name="persist", bufs=1))

        nc.scalar.add_instruction(mybir.InstLoadActFuncSet(
            name=f"I-{nc.next_id()}", ins=[], outs=[], act_func_set_id=6))

        ident = per.tile([128, 128], f32)
        masks.make_identity(nc, ident[:])
        ident_r = per.tile([128, 128], f32r)
        nc.vector.tensor_copy(out=ident_r[:], in_=ident[:])
        ones_f = per.tile([128, 1], f32)
        nc.vector.memset(ones_f[:], 1.0)
        ones_r = per.tile([128, 1], f32r)
        nc.vector.tensor_copy(out=ones_r[:], in_=ones_f[:])

        zn = per.tile([128, NT * 256], f32r)
        zT = per.tile([128, 2 * SIMW], f32r)
        exp_sb = per.tile([128, 4 * SIMW], f32r)
        dist_sb = per.tile([128, 4 * 400], f32r)
        sdist = per.tile([128, 12 * 100], f32r)
        norm2 = per.tile([128, NT], f32)
        lnn = per.tile([128, NT], f32)
        nn = per.tile([128, NT], f32)
        rn = per.tile([128, NT], f32)
        wt_sb = per.tile([128, 256], f32)
        w_sb = per.tile([128, 2 * 256], f32r)
        junk = per.tile([128, C], f32)
        out_sb = per.tile([128, 12], f32)
        cs_sb = per.tile([1, 1536], f32)
        nc.vector.memset(w_sb[:].bitcast(f32), 0.0)

        # ---------------- DMAs: 8 z chunks, then dist ----------------------
        for ch in range(NCH):
            nc.sync.dma_start(
                out=zn[:, ch * 1024:(ch + 1) * 1024].rearrange(
                    "p (q j) -> p q j", j=256),
                in_=z[ch * 512:(ch + 1) * 512, :].rearrange(
                    "(p q) j -> p q j", q=4),
            )
        nc.sync.dma_start(
            out=dist_sb[:].rearrange("p (ch q c) -> p ch q c", q=4, c=C),
            in_=dist.rearrange("(ch p q) c -> p ch q c", p=128, q=4),
        )

        # ---------------- PE warm-up: ramp tensor clock --------------------
        with tc.tile_pool(name="warm", bufs=1, space="PSUM") as wp:
            warm = wp.tile([128, 128], f32, tag="warm")
            for _ in range(16):
                nc.tensor.matmul(warm[:], lhsT=ident_r[:], rhs=ident_r[:],
                                 start=True, stop=True)

        sqd = ctx.enter_context(tc.tile_pool(name="sqd", bufs=3))

        def arrival(ch):
            return 2.9 + 1.46 * ch

        def norms_chunk(ch):
            for k in range(4):
                t = 4 * ch + k
                src = zn[:, t * 256:(t + 1) * 256].bitcast(f32)
                s = sqd.tile([128, 256], f32, tag="sq", name=f"sq{t}")
                nc.vector.scalar_tensor_tensor(
                    out=s[:], in0=src, scalar=1.0, in1=src,
                    op0=MULT, op1=MULT,
                    accum_out=norm2[:, t:t + 1])

        def rn_chain(t0, nt, need_nn):
            sl = slice(t0, t0 + nt)
            with tc.high_priority():
                nc.scalar.activation(lnn[:, sl], norm2[:, sl], LN)
                nc.scalar.activation(rn[:, sl], lnn[:, sl], EXP, scale=-0.5)
                if need_nn:
                    nc.vector.tensor_tensor(out=nn[:, sl], in0=norm2[:, sl],
                                            in1=rn[:, sl], op=MULT)

        def scale_chunk(ch):
            t0 = 4 * ch
            if ch >= 2:
                dst = zn[:, t0 * 256:(t0 + 2) * 256]
                bc = rn[:, t0:t0 + 2].unsqueeze(2).to_broadcast([128, 2, 256])
                nc.vector.scalar_tensor_tensor(
                    out=dst.rearrange("p (k j) -> p k j", j=256),
                    in0=dst.bitcast(f32).rearrange("p (k j) -> p k j", j=256),
                    scalar=1.0, in1=bc, op0=MULT, op1=MULT)
                ks = (2, 3)
            else:
                ks = (0, 1, 2, 3)
            for k in ks:
                t = 4 * ch + k
                d2 = zn[:, t * 256:(t + 1) * 256]
                nc.gpsimd.normalize_recip(
                    out_ap=d2, in_ap=d2.bitcast(f32),
                    denom_ap=nn[:, t:t + 1])

        def transpose_chunk(ptp, ch):
            for d in range(2):
                pt = ptp.tile([128, 512], f32r, tag="tr", name=f"pt{ch}_{d}")
                for k in range(4):
                    t = 4 * ch + k
                    nc.tensor.transpose(
                        pt[:, k * 128:(k + 1) * 128],
                        zn[:, t * 256 + d * 128: t * 256 + d * 128 + 128],
                        ident_r[:],
                    )
                dst = zT[:, d * SIMW + ch * 512: d * SIMW + (ch + 1) * 512]
                nc.scalar.copy(out=dst, in_=pt[:])

        def w_tiles(ts):
            for t in ts:
                if t < NSC:
                    lhsT = dist_sb[:, (t // 4 % 4) * 400 + (t % 4) * 100:
                                   (t // 4 % 4) * 400 + (t % 4) * 100 + 100]
                else:
                    lhsT = sdist[:, (t - NSC) * 100:(t - NSC + 1) * 100]
                nc.tensor.matmul(
                    wt_ps[0:C, :], lhsT=lhsT,
                    rhs=zn[:, t * 256:(t + 1) * 256],
                    start=(t == 0), stop=(t == NT - 1),
                )

        gA = ctx.enter_context(tc.tile_pool(name="gA", bufs=1, space="PSUM"))
        gB = ctx.enter_context(tc.tile_pool(name="gB", bufs=1, space="PSUM"))
        pw_cm = tc.tile_pool(name="pw", bufs=1, space="PSUM")
        pw = pw_cm.__enter__()
        wt_ps = pw.tile([128, 256], f32, tag="wt", name="wt_ps")

        def sims_into(g, rt, b, col0):
            for d in range(2):
                nc.tensor.matmul(
                    g[:, col0:col0 + 512],
                    lhsT=zT[:, d * SIMW + rt * 128: d * SIMW + (rt + 1) * 128],
                    rhs=zT[:, d * SIMW + b * 512: d * SIMW + (b + 1) * 512],
                    start=(d == 0), stop=(d == 1),
                )

        def exp_granule(g, rt, gi, width):
            nc.scalar.activation(
                out=exp_sb[:, rt * SIMW + gi * 1024:
                           rt * SIMW + gi * 1024 + width],
                in_=g[:, 0:width], func=EXP,
                accum_out=out_sb[:, gi * 4 + rt: gi * 4 + rt + 1])

        def granules_for(rt):
            a = gA.tile([128, 1024], f32, tag="gA", name=f"g{rt}a")
            sims_into(a, rt, 0, 0)
            sims_into(a, rt, 1, 512)
            exp_granule(a, rt, 0, 1024)
            bb = gB.tile([128, 1536], f32, tag="gB", name=f"g{rt}b")
            for b in (2, 3, 4):
                sims_into(bb, rt, b, (b - 2) * 512)
            exp_granule(bb, rt, 1, 1536)

        # ---------------- chunks 0-4 + strip rt0 granules -------------------
        with tc.tile_pool(name="pt", bufs=2, space="PSUM") as ptp:
            a0 = b0 = None
            for ch in range(NBLK):
                tc.tile_set_cur_wait(arrival(ch) / 1000.0)
                norms_chunk(ch)
                tc.tile_set_cur_wait((arrival(ch) + 0.40) / 1000.0)
                rn_chain(4 * ch, 4, need_nn=True)
                tc.tile_set_cur_wait((arrival(ch) + 0.75) / 1000.0)
                scale_chunk(ch)
                tc.tile_set_cur_wait((arrival(ch) + 1.30) / 1000.0)
                transpose_chunk(ptp, ch)
                tc.tile_set_cur_wait((arrival(ch) + 1.95) / 1000.0)
                if ch == 0:
                    a0 = gA.tile([128, 1024], f32, tag="gA", name="g0a")
                    sims_into(a0, 0, 0, 0)
                elif ch == 1:
                    sims_into(a0, 0, 1, 512)
                    exp_granule(a0, 0, 0, 1024)
                elif ch == 2:
                    b0 = gB.tile([128, 1536], f32, tag="gB", name="g0b")
                    sims_into(b0, 0, 2, 0)
                elif ch == 3:
                    sims_into(b0, 0, 3, 512)
                else:
                    sims_into(b0, 0, 4, 1024)
                    exp_granule(b0, 0, 1, 1536)

        for ch in range(NBLK, NCH):
            tc.tile_set_cur_wait(arrival(ch) / 1000.0)
            norms_chunk(ch)
        tc.tile_set_cur_wait((arrival(7) + 0.40) / 1000.0)
        rn_chain(NSC, 12, need_nn=True)
        for t in range(NSC, NT):
            srcd = dist_sb[:, (t // 4 % 4) * 400 + (t % 4) * 100:
                           (t // 4 % 4) * 400 + (t % 4) * 100 + 100]
            nc.gpsimd.normalize_recip(
                out_ap=sdist[:, (t - NSC) * 100:(t - NSC + 1) * 100],
                in_ap=srcd.bitcast(f32),
                denom_ap=nn[:, t:t + 1])

        # ---------------- strips 1-3, W tail, P path ------------------------
        tc.tile_set_cur_wait(12.0 / 1000.0)
        w_tiles(range(0, NSC))
        granules_for(1)
        tc.tile_set_cur_wait((arrival(7) + 1.5) / 1000.0)
        w_tiles(range(NSC, NT))
        granules_for(2)

        nc.vector.tensor_copy(out=wt_sb[0:C, :], in_=wt_ps[0:C, :])
        for d in range(2):
            w_ps = pw.tile([128, 128], f32, tag="wt", name=f"w_ps{d}")
            nc.tensor.transpose(
                w_ps[:, 0:C],
                wt_sb[0:C, d * 128:(d + 1) * 128],
                ident[0:C, 0:C],
            )
            nc.vector.tensor_copy(out=w_sb[:, d * 256:d * 256 + C],
                                  in_=w_ps[:, 0:C])

        bb3 = gB.tile([128, 1536], f32, tag="gB", name="g3b")
        for b in (2, 3, 4):
            sims_into(bb3, 3, b, (b - 2) * 512)
        exp_granule(bb3, 3, 1, 1536)
        a3 = gA.tile([128, 1024], f32, tag="gA", name="g3a")
        sims_into(a3, 3, 0, 0)
        sims_into(a3, 3, 1, 512)
        exp_granule(a3, 3, 0, 1024)

        for rt in range(4):
            q_ps = pw.tile([128, 256], f32, tag="wt", name=f"q_ps{rt}")
            for d in range(2):
                nc.tensor.matmul(
                    q_ps[:], lhsT=zT[:, d * SIMW + rt * 128:
                                     d * SIMW + (rt + 1) * 128],
                    rhs=w_sb[:, d * 256:(d + 1) * 256],
                    start=(d == 0), stop=(d == 1),
                )
            nc.vector.scalar_tensor_tensor(
                out=junk[:], in0=q_ps[:, 0:C], scalar=1.0,
                in1=dist_sb[:, rt * 100:rt * 100 + 100].bitcast(f32),
                op0=MULT, op1=MULT,
                accum_out=out_sb[:, 8 + rt:9 + rt])
        pw_cm.__exit__(None, None, None)

        # ---------------- column sums of blocks B1-B3 -----------------------
        with tc.tile_pool(name="cs", bufs=1, space="PSUM") as csp:
            cs_ps = csp.tile([128, 1536], f32, tag="cs")
            for rt in range(4):
                for k in (1, 2, 3):
                    nc.tensor.matmul(
                        cs_ps[0:1, (k - 1) * 512:k * 512], lhsT=ones_r[:],
                        rhs=exp_sb[:, rt * SIMW + k * 512:
                                   rt * SIMW + (k + 1) * 512],
                        start=(rt == 0), stop=(rt == 3),
                    )
            nc.scalar.copy(out=cs_sb[:, 0:512], in_=cs_ps[0:1, 0:512])
            nc.vector.tensor_copy(out=cs_sb[:, 512:1024],
                                  in_=cs_ps[0:1, 512:1024])
            nc.scalar.copy(out=cs_sb[:, 1024:1536],
                           in_=cs_ps[0:1, 1024:1536])
            nc.sync.dma_start(out=out2[:], in_=cs_sb[:])

        nc.sync.dma_start(out=out[:], in_=out_sb[:])

    nc.finalize()
    return nc


def _get_program():
    global _PROG
    if _PROG is None:
        _PROG = _build_program()
    return _PROG


def kernel(z_i, z_j, z_n, dist_labels):
    from concourse.bass_utils import run_bass_kernel_spmd

    nc = _get_program()

    z_full = np.ascontiguousarray(
        np.concatenate([z_i, z_j], axis=0), dtype=np.float32
    )
    dist = np.ascontiguousarray(dist_labels, dtype=np.float32)

    in_maps = []
    for c in range(N_CORES):
        r0 = c * RPC
        in_maps.append({
            "z": np.ascontiguousarray(np.roll(z_full, -r0, axis=0)),
            "dist": np.ascontiguousarray(np.roll(dist, -r0, axis=0)),
        })

    res = run_bass_kernel_spmd(nc, in_maps, list(range(N_CORES))).results

    S = np.zeros(N, np.float64)
    P = np.empty(N, np.float64)
    idx = np.arange(RPC)
    for c in range(N_CORES):
        o = res[c]["out"].astype(np.float64)
        g = (idx + RPC * c) % N
        # device order p*4+rt == local row 4p+rt
        S[g] += (o[:, 0:4] + o[:, 4:8]).reshape(RPC)
        P[g] = o[:, 8:12].reshape(RPC)
        cs = res[c]["out2"].astype(np.float64).reshape(3, 4, 128)
        for k in (1, 2, 3):
            # block col j=kk*128+p -> local row 512k + 4p + kk
            cs_r = cs[k - 1].T.reshape(RPC)
            gk = (idx + RPC * (c + k)) % N
            S[gk] += cs_r

    S -= E_CONST
    P -= 1.0
    return np.float32((P / S).sum() / N)


# revision 29
# speedup vs baseline: 1.2439x; 1.0193x over previous
"""NT-Xent loss kernel, v19: symmetric-block decomposition (38.5us
cost-model vs 47.9us baseline; rel err ~5e-3).

Decomposition: after the host-side rotation every core's q rows are
local rows 0-511; each core computes sim blocks (q, B0..B4) = local
cols 0..2559 only (5 of 8 column blocks). Row sums come from the exp
accumulators; the missing col groups m=5,6,7 for each row are COLUMN
sums of blocks B3,B2,B1 computed on cores i+5,i+6,i+7 (sim symmetry),
taken with ones-vector matmuls over the retained f32r exp values and
combined on the host. B4 pairs with core i+4's B4 (computed twice
fleet-wide, row sums only). This cuts sim matmuls 13.7->8.5us on PE
and exp work 17->11.5us on ACT vs the full-row v11 design.

Schedule/assignment (found by cost-model iteration):
- 4-rows-per-partition interleave: partition p of tile t=4ch+k holds
  local row 512ch+4p+k -> dist DMA descriptors are 1600B contiguous
  (2.28us vs 4.55us) and q/dist tiles line up for the P path.
- norms: one scalar_tensor_tensor (x*1)*x with accum_out per tile,
  all on DVE (GPSIMD rejects generic tensor ops at codegen).
- rn = exp(-0.5 ln norm2) on ACT; nn = norm2*rn on DVE.
- scale rows 0..2559 in place: ch0-1 all-Pool normalize_recip; ch2-4
  k0,k1 via one DVE broadcast-STT + k2,k3 Pool. Rows 2560+ instead
  fold rn into dist (sdist, Pool nrecip); W uses raw z there.
- transposes on PE; PSUM->SBUF copies on ACT.
- sims land in PSUM granules A=[128,1024] (b0,b1) + B=[128,1536]
  (b2-b4) per q-tile, ONE exp+accum each straight into the output
  tile; granule pools let PE run ahead of ACT.
- W = dist^T z_hat (32 matmuls) issued after the chunk loop to fill
  PE gaps during the exp phase; P path q_ps padded to 256 free
  (1 cyc/row) with the mask-multiply+reduce fused into one STT.
- colsum matmuls accumulate over the 4 q-tiles in a 3-bank PSUM
  window after the W pool closes; staged to SBUF and DMA'd as out2.
- PE warm-up matmuls ramp the tensor clock before the transposes.
- tile_set_cur_wait floors stage the scheduler per chunk arrival
  (DMA sem overhead ~0.9us included).

Engine legality learned on hardware: GPSIMD supports only its custom
ISA ops (normalize_recip etc) and cannot access PSUM; activation table
set 6 = {ln, exp, square, copy, ...}; Rsqrt/Reciprocal are banned;
f32r matmul inputs must be produced typed f32r (DVE/ACT writes
qualify); transpose-mode matmul needs a permutation rhs;
tensor_tensor_reduce crashes the exec unit.
"""

import numpy as np
from contextlib import ExitStack

N = 4096
D = 256
C = 100
B = 2048
N_CORES = 8
RPC = 512
NT = 32
NCH = 8
NBLK = 5
SIMW = NBLK * 512
NSC = NBLK * 4
E_CONST = float(np.e)

_PROG = None


def _build_program():
    import concourse.bass as bass
    import concourse.tile as tile
    from concourse import bacc, mybir, masks

    f32 = mybir.dt.float32
    f32r = mybir.dt.float32r
    MULT = mybir.AluOpType.mult
    EXP = mybir.ActivationFunctionType.Exp
    LN = mybir.ActivationFunctionType.Ln

    nc = bacc.Bacc(
        "TRN2",
        target_bir_lowering=False,
        debug=False,
        enable_asserts=False,
        num_devices=N_CORES,
    )

    z = nc.dram_tensor("z", [N, D], f32r, kind="ExternalInput").ap()
    dist = nc.dram_tensor("dist", [B, C], f32r, kind="ExternalInput").ap()
    out = nc.dram_tensor("out", [128, 12], f32, kind="ExternalOutput").ap()
    out2 = nc.dram_tensor("out2", [1, 1536], f32, kind="ExternalOutput").ap()

    with tile.TileContext(nc) as tc, ExitStack() as ctx:
        per = ctx.enter_context(tc.tile_pool(name="persist", bufs=1))

        nc.scalar.add_instruction(mybir.InstLoadActFuncSet(
            name=f"I-{nc.next_id()}", ins=[], outs=[], act_func_set_id=6))

        ident = per.tile([128, 128], f32)
        masks.make_identity(nc, ident[:])
        ident_r = per.tile([128, 128], f32r)
        nc.vector.tensor_copy(out=ident_r[:], in_=ident[:])
        ones_f = per.tile([128, 1], f32)
        nc.vector.memset(ones_f[:], 1.0)
        ones_r = per.tile([128, 1], f32r)
        nc.vector.tensor_copy(out=ones_r[:], in_=ones_f[:])

        zn = per.tile([128, NT * 256], f32r)
        zT = per.tile([128, 2 * SIMW], f32r)
        exp_sb = per.tile([128, 4 * SIMW], f32r)
        dist_sb = per.tile([128, 4 * 400], f32r)
        sdist = per.tile([128, 12 * 100], f32r)
        norm2 = per.tile([128, NT], f32)
        lnn = per.tile([128, NT], f32)
        nn = per.tile([128, NT], f32)
        rn = per.tile([128, NT], f32)
        wt_sb = per.tile([128, 256], f32)
        w_sb = per.tile([128, 2 * 256], f32r)
        junk = per.tile([128, C], f32)
        out_sb = per.tile([128, 12], f32)
        cs_sb = per.tile([1, 1536], f32)
        nc.vector.memset(w_sb[:].bitcast(f32), 0.0)

        # ---------------- DMAs: 8 z chunks, then dist ----------------------
        for ch in range(NCH):
            nc.sync.dma_start(
                out=zn[:, ch * 1024:(ch + 1) * 1024].rearrange(
                    "p (q j) -> p q j", j=256),
                in_=z[ch * 512:(ch + 1) * 512, :].rearrange(
                    "(p q) j -> p q j", q=4),
            )
        nc.sync.dma_start(
            out=dist_sb[:].rearrange("p (ch q c) -> p ch q c", q=4, c=C),
            in_=dist.rearrange("(ch p q) c -> p ch q c", p=128, q=4),
        )

        # ---------------- PE warm-up: ramp tensor clock --------------------
        with tc.tile_pool(name="warm", bufs=1, space="PSUM") as wp:
            warm = wp.tile([128, 128], f32, tag="warm")
            for _ in range(16):
                nc.tensor.matmul(warm[:], lhsT=ident_r[:], rhs=ident_r[:],
                                 start=True, stop=True)

        sqd = ctx.enter_context(tc.tile_pool(name="sqd", bufs=3))

        def arrival(ch):
            return 2.9 + 1.46 * ch

        def norms_chunk(ch):
            for k in range(4):
                t = 4 * ch + k
                src = zn[:, t * 256:(t + 1) * 256].bitcast(f32)
                s = sqd.tile([128, 256], f32, tag="sq", name=f"sq{t}")
                nc.vector.scalar_tensor_tensor(
                    out=s[:], in0=src, scalar=1.0, in1=src,
                    op0=MULT, op1=MULT,
                    accum_out=norm2[:, t:t + 1])

        def rn_chain(t0, nt, need_nn):
            sl = slice(t0, t0 + nt)
            with tc.high_priority():
                nc.scalar.activation(lnn[:, sl], norm2[:, sl], LN)
                nc.scalar.activation(rn[:, sl], lnn[:, sl], EXP, scale=-0.5)
                if need_nn:
                    nc.vector.tensor_tensor(out=nn[:, sl], in0=norm2[:, sl],
                                            in1=rn[:, sl], op=MULT)

        def scale_chunk(ch):
            t0 = 4 * ch
            if ch >= 2:
                dst = zn[:, t0 * 256:(t0 + 2) * 256]
                bc = rn[:, t0:t0 + 2].unsqueeze(2).to_broadcast([128, 2, 256])
                nc.vector.scalar_tensor_tensor(
                    out=dst.rearrange("p (k j) -> p k j", j=256),
                    in0=dst.bitcast(f32).rearrange("p (k j) -> p k j", j=256),
                    scalar=1.0, in1=bc, op0=MULT, op1=MULT)
                ks = (2, 3)
            else:
                ks = (0, 1, 2, 3)
            for k in ks:
                t = 4 * ch + k
                d2 = zn[:, t * 256:(t + 1) * 256]
                nc.gpsimd.normalize_recip(
                    out_ap=d2, in_ap=d2.bitcast(f32),
                    denom_ap=nn[:, t:t + 1])

        def transpose_chunk(ptp, ch):
            for d in range(2):
                pt = ptp.tile([128, 512], f32r, tag="tr", name=f"pt{ch}_{d}")
                for k in range(4):
                    t = 4 * ch + k
                    nc.tensor.transpose(
                        pt[:, k * 128:(k + 1) * 128],
                        zn[:, t * 256 + d * 128: t * 256 + d * 128 + 128],
                        ident_r[:],
                    )
                dst = zT[:, d * SIMW + ch * 512: d * SIMW + (ch + 1) * 512]
                nc.scalar.copy(out=dst, in_=pt[:])

        def w_tiles(ts):
            for t in ts:
                if t < NSC:
                    lhsT = dist_sb[:, (t // 4 % 4) * 400 + (t % 4) * 100:
                                   (t // 4 % 4) * 400 + (t % 4) * 100 + 100]
                else:
                    lhsT = sdist[:, (t - NSC) * 100:(t - NSC + 1) * 100]
                nc.tensor.matmul(
                    wt_ps[0:C, :], lhsT=lhsT,
                    rhs=zn[:, t * 256:(t + 1) * 256],
                    start=(t == 0), stop=(t == NT - 1),
                )

        gA = ctx.enter_context(tc.tile_pool(name="gA", bufs=1, space="PSUM"))
        gB = ctx.enter_context(tc.tile_pool(name="gB", bufs=1, space="PSUM"))
        pw_cm = tc.tile_pool(name="pw", bufs=1, space="PSUM")
        pw = pw_cm.__enter__()
        wt_ps = pw.tile([128, 256], f32, tag="wt", name="wt_ps")

        def sims_into(g, rt, b, col0):
            for d in range(2):
                nc.tensor.matmul(
                    g[:, col0:col0 + 512],
                    lhsT=zT[:, d * SIMW + rt * 128: d * SIMW + (rt + 1) * 128],
                    rhs=zT[:, d * SIMW + b * 512: d * SIMW + (b + 1) * 512],
                    start=(d == 0), stop=(d == 1),
                )

        def exp_granule(g, rt, gi, width):
            nc.scalar.activation(
                out=exp_sb[:, rt * SIMW + gi * 1024:
                           rt * SIMW + gi * 1024 + width],
                in_=g[:, 0:width], func=EXP,
                accum_out=out_sb[:, gi * 4 + rt: gi * 4 + rt + 1])

        def granules_for(rt):
            a = gA.tile([128, 1024], f32, tag="gA", name=f"g{rt}a")
            sims_into(a, rt, 0, 0)
            sims_into(a, rt, 1, 512)
            exp_granule(a, rt, 0, 1024)
            bb = gB.tile([128, 1536], f32, tag="gB", name=f"g{rt}b")
            for b in (2, 3, 4):
                sims_into(bb, rt, b, (b - 2) * 512)
            exp_granule(bb, rt, 1, 1536)

        # ---------------- chunks 0-4 + strip rt0 granules -------------------
        with tc.tile_pool(name="pt", bufs=2, space="PSUM") as ptp:
            a0 = b0 = None
            for ch in range(NBLK):
                tc.tile_set_cur_wait(arrival(ch) / 1000.0)
                norms_chunk(ch)
                tc.tile_set_cur_wait((arrival(ch) + 0.40) / 1000.0)
                rn_chain(4 * ch, 4, need_nn=True)
                tc.tile_set_cur_wait((arrival(ch) + 0.75) / 1000.0)
                scale_chunk(ch)
                tc.tile_set_cur_wait((arrival(ch) + 1.30) / 1000.0)
                transpose_chunk(ptp, ch)
                tc.tile_set_cur_wait((arrival(ch) + 1.95) / 1000.0)
                if ch == 0:
                    a0 = gA.tile([128, 1024], f32, tag="gA", name="g0a")
                    sims_into(a0, 0, 0, 0)
                elif ch == 1:
                    sims_into(a0, 0, 1, 512)
                    exp_granule(a0, 0, 0, 1024)
                elif ch == 2:
                    b0 = gB.tile([128, 1536], f32, tag="gB", name="g0b")
                    sims_into(b0, 0, 2, 0)
                elif ch == 3:
                    sims_into(b0, 0, 3, 512)
                else:
                    sims_into(b0, 0, 4, 1024)
                    exp_granule(b0, 0, 1, 1536)

        for ch in range(NBLK, NCH):
            tc.tile_set_cur_wait(arrival(ch) / 1000.0)
            norms_chunk(ch)
        tc.tile_set_cur_wait((arrival(7) + 0.40) / 1000.0)
        rn_chain(NSC, 12, need_nn=True)
        for t in range(NSC, NT):
            srcd = dist_sb[:, (t // 4 % 4) * 400 + (t % 4) * 100:
                           (t // 4 % 4) * 400 + (t % 4) * 100 + 100]
            nc.gpsimd.normalize_recip(
                out_ap=sdist[:, (t - NSC) * 100:(t - NSC + 1) * 100],
                in_ap=srcd.bitcast(f32),
                denom_ap=nn[:, t:t + 1])

        # ---------------- strips 1-3, W tail, P path ------------------------
        tc.tile_set_cur_wait(12.0 / 1000.0)
        w_tiles(range(0, NSC))
        granules_for(1)
        tc.tile_set_cur_wait((arrival(7) + 1.5) / 1000.0)
        w_tiles(range(NSC, NT))
        granules_for(2)

        nc.vector.tensor_copy(out=wt_sb[0:C, :], in_=wt_ps[0:C, :])
        for d in range(2):
            w_ps = pw.tile([128, 128], f32, tag="wt", name=f"w_ps{d}")
            nc.tensor.transpose(
                w_ps[:, 0:C],
                wt_sb[0:C, d * 128:(d + 1) * 128],
                ident[0:C, 0:C],
            )
            nc.vector.tensor_copy(out=w_sb[:, d * 256:d * 256 + C],
                                  in_=w_ps[:, 0:C])

        bb3 = gB.tile([128, 1536], f32, tag="gB", name="g3b")
        for b in (2, 3, 4):
            sims_into(bb3, 3, b, (b - 2) * 512)
        exp_granule(bb3, 3, 1, 1536)
        a3 = gA.tile([128, 1024], f32, tag="gA", name="g3a")
        sims_into(a3, 3, 0, 0)
        sims_into(a3, 3, 1, 512)
        exp_granule(a3, 3, 0, 1024)

        for rt in range(4):
            q_ps = pw.tile([128, 256], f32, tag="wt", name=f"q_ps{rt}")
            for d in range(2):
                nc.tensor.matmul(
                    q_ps[:], lhsT=zT[:, d * SIMW + rt * 128:
                                     d * SIMW + (rt + 1) * 128],
                    rhs=w_sb[:, d * 256:(d + 1) * 256],
                    start=(d == 0), stop=(d == 1),
                )
            nc.vector.scalar_tensor_tensor(
                out=junk[:], in0=q_ps[:, 0:C], scalar=1.0,
                in1=dist_sb[:, rt * 100:rt * 100 + 100].bitcast(f32),
                op0=MULT, op1=MULT,
                accum_out=out_sb[:, 8 + rt:9 + rt])
        pw_cm.__exit__(None, None, None)

        # ---------------- column sums of blocks B1-B3 -----------------------
        with tc.tile_pool(name="cs", bufs=1, space="PSUM") as csp:
            cs_ps = csp.tile([128, 1536], f32, tag="cs")
            for rt in range(4):
                for k in (1, 2, 3):
                    nc.tensor.matmul(
                        cs_ps[0:1, (k - 1) * 512:k * 512], lhsT=ones_r[:],
                        rhs=exp_sb[:, rt * SIMW + k * 512:
                                   rt * SIMW + (k + 1) * 512],
                        start=(rt == 0), stop=(rt == 3),
                    )
            nc.scalar.copy(out=cs_sb[:, 0:512], in_=cs_ps[0:1, 0:512])
            nc.vector.tensor_copy(out=cs_sb[:, 512:1024],
                                  in_=cs_ps[0:1, 512:1024])
            nc.scalar.copy(out=cs_sb[:, 1024:1536],
                           in_=cs_ps[0:1, 1024:1536])
            nc.sync.dma_start(out=out2[:], in_=cs_sb[:])

        nc.sync.dma_start(out=out[:], in_=out_sb[:])

    nc.finalize()
    return nc


def _get_program():
    global _PROG
    if _PROG is None:
        _PROG = _build_program()
    return _PROG


def kernel(z_i, z_j, z_n, dist_labels):
    from concourse.bass_utils import run_bass_kernel_spmd

    nc = _get_program()

    z_full = np.ascontiguousarray(
        np.concatenate([z_i, z_j], axis=0), dtype=np.float32
    )
    dist = np.ascontiguousarray(dist_labels, dtype=np.float32)

    in_maps = []
    for c in range(N_CORES):
        r0 = c * RPC
        in_maps.append({
            "z": np.ascontiguousarray(np.roll(z_full, -r0, axis=0)),
            "dist": np.ascontiguousarray(np.roll(dist, -r0, axis=0)),
        })

    res = run_bass_kernel_spmd(nc, in_maps, list(range(N_CORES))).results

    S = np.zeros(N, np.float64)
    P = np.empty(N, np.float64)
    idx = np.arange(RPC)
    for c in range(N_CORES):
        o = res[c]["out"].astype(np.float64)
        g = (idx + RPC * c) % N
        # device order p*4+rt == local row 4p+rt
        S[g] += (o[:, 0:4] + o[:, 4:8]).reshape(RPC)
        P[g] = o[:, 8:12].reshape(RPC)
        cs = res[c]["out2"].astype(np.float64).reshape(3, 4, 128)
        for k in (1, 2, 3):
            # block col j=kk*128+p -> local row 512k + 4p + kk
            cs_r = cs[k - 1].T.reshape(RPC)
            gk = (idx + RPC * (c + k)) % N
            S[gk] += cs_r

    S -= E_CONST
    P -= 1.0
    return np.float32((P / S).sum() / N)


# revision 43
# speedup vs baseline: 1.2984x; 1.0438x over previous
"""NT-Xent loss kernel, v20: symmetric-block decomposition (36.9us
cost-model vs 47.9us baseline; rel err ~5e-3).

v20 vs v19: sims land in FOUR independent single-buffer PSUM granule
rings per q-tile (A=b0,b1 [128,1024]; B1=b2, B2=b3, C=b4 [128,512]
each, 5 banks total) instead of two. Each granule gets its own
exp+accum into a distinct output column (host sums the four partial
row-sum groups). Finer rings decouple PE from ACT: the next strip's
sims for one 512-col block only wait for THAT block's exp, so the
exp->sims->exp serialization cycle shrinks and ACT runs ~69%% busy
through the endgame. W matmuls are issued after the chunk loop
(floor 12us) to fill the remaining PE gaps.

Decomposition: after the host-side rotation every core's q rows are
local rows 0-511; each core computes sim blocks (q, B0..B4) = local
cols 0..2559 only (5 of 8 column blocks). Row sums come from the exp
accumulators; the missing col groups m=5,6,7 for each row are COLUMN
sums of blocks B3,B2,B1 computed on cores i+5,i+6,i+7 (sim symmetry),
taken with ones-vector matmuls over the retained f32r exp values and
combined on the host. B4 pairs with core i+4's B4 (computed twice
fleet-wide, row sums only). This cuts sim matmuls 13.7->8.5us on PE
and exp work 17->11.5us on ACT vs the full-row v11 design.

Schedule/assignment (found by cost-model iteration):
- 4-rows-per-partition interleave: partition p of tile t=4ch+k holds
  local row 512ch+4p+k -> dist DMA descriptors are 1600B contiguous
  (2.28us vs 4.55us) and q/dist tiles line up for the P path.
- norms: one scalar_tensor_tensor (x*1)*x with accum_out per tile,
  all on DVE (GPSIMD rejects generic tensor ops at codegen).
- rn = exp(-0.5 ln norm2) on ACT; nn = norm2*rn on DVE.
- scale rows 0..2559 in place: ch0-1 all-Pool normalize_recip; ch2-4
  k0,k1 via one DVE broadcast-STT + k2,k3 Pool. Rows 2560+ instead
  fold rn into dist (sdist, Pool nrecip); W uses raw z there.
- transposes on PE; PSUM->SBUF copies on ACT.
- sims land in PSUM granules A=[128,1024] (b0,b1) + B=[128,1536]
  (b2-b4) per q-tile, ONE exp+accum each straight into the output
  tile; granule pools let PE run ahead of ACT.
- W = dist^T z_hat (32 matmuls) issued after the chunk loop to fill
  PE gaps during the exp phase; P path q_ps padded to 256 free
  (1 cyc/row) with the mask-multiply+reduce fused into one STT.
- colsum matmuls accumulate over the 4 q-tiles in a 3-bank PSUM
  window after the W pool closes; staged to SBUF and DMA'd as out2.
- PE warm-up matmuls ramp the tensor clock before the transposes.
- tile_set_cur_wait floors stage the scheduler per chunk arrival
  (DMA sem overhead ~0.9us included).

Engine legality learned on hardware: GPSIMD supports only its custom
ISA ops (normalize_recip etc) and cannot access PSUM; activation table
set 6 = {ln, exp, square, copy, ...}; Rsqrt/Reciprocal are banned;
f32r matmul inputs must be produced typed f32r (DVE/ACT writes
qualify); transpose-mode matmul needs a permutation rhs;
tensor_tensor_reduce crashes the exec unit.
"""

import numpy as np
from contextlib import ExitStack

N = 4096
D = 256
C = 100
B = 2048
N_CORES = 8
RPC = 512
NT = 32
NCH = 8
NBLK = 5
SIMW = NBLK * 512
NSC = NBLK * 4
E_CONST = float(np.e)

_PROG = None


def _build_program():
    import concourse.bass as bass
    import concourse.tile as tile
    from concourse import bacc, mybir, masks

    f32 = mybir.dt.float32
    f32r = mybir.dt.float32r
    MULT = mybir.AluOpType.mult
    EXP = mybir.ActivationFunctionType.Exp
    LN = mybir.ActivationFunctionType.Ln

    nc = bacc.Bacc(
        "TRN2",
        target_bir_lowering=False,
        debug=False,
        enable_asserts=False,
        num_devices=N_CORES,
    )

    z = nc.dram_tensor("z", [N, D], f32r, kind="ExternalInput").ap()
    dist = nc.dram_tensor("dist", [B, C], f32r, kind="ExternalInput").ap()
    out = nc.dram_tensor("out", [128, 20], f32, kind="ExternalOutput").ap()
    out2 = nc.dram_tensor("out2", [1, 1536], f32, kind="ExternalOutput").ap()

    with tile.TileContext(nc) as tc, ExitStack() as ctx:
        per = ctx.enter_context(tc.tile_pool(name="persist", bufs=1))

        nc.scalar.add_instruction(mybir.InstLoadActFuncSet(
            name=f"I-{nc.next_id()}", ins=[], outs=[], act_func_set_id=6))

        ident = per.tile([128, 128], f32)
        masks.make_identity(nc, ident[:])
        ident_r = per.tile([128, 128], f32r)
        nc.vector.tensor_copy(out=ident_r[:], in_=ident[:])
        ones_f = per.tile([128, 1], f32)
        nc.vector.memset(ones_f[:], 1.0)
        ones_r = per.tile([128, 1], f32r)
        nc.vector.tensor_copy(out=ones_r[:], in_=ones_f[:])

        zn = per.tile([128, NT * 256], f32r)
        zT = per.tile([128, 2 * SIMW], f32r)
        exp_sb = per.tile([128, 4 * SIMW], f32r)
        dist_sb = per.tile([128, 4 * 400], f32r)
        sdist = per.tile([128, 12 * 100], f32r)
        norm2 = per.tile([128, NT], f32)
        lnn = per.tile([128, NT], f32)
        nn = per.tile([128, NT], f32)
        rn = per.tile([128, NT], f32)
        wt_sb = per.tile([128, 256], f32)
        w_sb = per.tile([128, 2 * 256], f32r)
        junk = per.tile([128, C], f32)
        out_sb = per.tile([128, 20], f32)
        cs_sb = per.tile([1, 1536], f32)
        nc.vector.memset(w_sb[:].bitcast(f32), 0.0)

        # ---------------- DMAs: 8 z chunks, then dist ----------------------
        for ch in range(NCH):
            nc.sync.dma_start(
                out=zn[:, ch * 1024:(ch + 1) * 1024].rearrange(
                    "p (q j) -> p q j", j=256),
                in_=z[ch * 512:(ch + 1) * 512, :].rearrange(
                    "(p q) j -> p q j", q=4),
            )
        nc.sync.dma_start(
            out=dist_sb[:].rearrange("p (ch q c) -> p ch q c", q=4, c=C),
            in_=dist.rearrange("(ch p q) c -> p ch q c", p=128, q=4),
        )

        # ---------------- PE warm-up: ramp tensor clock --------------------
        with tc.tile_pool(name="warm", bufs=1, space="PSUM") as wp:
            warm = wp.tile([128, 128], f32, tag="warm")
            for _ in range(16):
                nc.tensor.matmul(warm[:], lhsT=ident_r[:], rhs=ident_r[:],
                                 start=True, stop=True)

        sqd = ctx.enter_context(tc.tile_pool(name="sqd", bufs=3))

        def arrival(ch):
            return 2.9 + 1.46 * ch

        def norms_chunk(ch):
            for k in range(4):
                t = 4 * ch + k
                src = zn[:, t * 256:(t + 1) * 256].bitcast(f32)
                s = sqd.tile([128, 256], f32, tag="sq", name=f"sq{t}")
                nc.vector.scalar_tensor_tensor(
                    out=s[:], in0=src, scalar=1.0, in1=src,
                    op0=MULT, op1=MULT,
                    accum_out=norm2[:, t:t + 1])

        def rn_chain(t0, nt, need_nn):
            sl = slice(t0, t0 + nt)
            with tc.high_priority():
                nc.scalar.activation(lnn[:, sl], norm2[:, sl], LN)
                nc.scalar.activation(rn[:, sl], lnn[:, sl], EXP, scale=-0.5)
                if need_nn:
                    nc.vector.tensor_tensor(out=nn[:, sl], in0=norm2[:, sl],
                                            in1=rn[:, sl], op=MULT)

        def scale_chunk(ch):
            t0 = 4 * ch
            if ch >= 2:
                dst = zn[:, t0 * 256:(t0 + 2) * 256]
                bc = rn[:, t0:t0 + 2].unsqueeze(2).to_broadcast([128, 2, 256])
                nc.vector.scalar_tensor_tensor(
                    out=dst.rearrange("p (k j) -> p k j", j=256),
                    in0=dst.bitcast(f32).rearrange("p (k j) -> p k j", j=256),
                    scalar=1.0, in1=bc, op0=MULT, op1=MULT)
                ks = (2, 3)
            else:
                ks = (0, 1, 2, 3)
            for k in ks:
                t = 4 * ch + k
                d2 = zn[:, t * 256:(t + 1) * 256]
                nc.gpsimd.normalize_recip(
                    out_ap=d2, in_ap=d2.bitcast(f32),
                    denom_ap=nn[:, t:t + 1])

        def transpose_chunk(ptp, ch):
            for d in range(2):
                pt = ptp.tile([128, 512], f32r, tag="tr", name=f"pt{ch}_{d}")
                for k in range(4):
                    t = 4 * ch + k
                    nc.tensor.transpose(
                        pt[:, k * 128:(k + 1) * 128],
                        zn[:, t * 256 + d * 128: t * 256 + d * 128 + 128],
                        ident_r[:],
                    )
                dst = zT[:, d * SIMW + ch * 512: d * SIMW + (ch + 1) * 512]
                nc.scalar.copy(out=dst, in_=pt[:])

        def w_tiles(ts):
            for t in ts:
                if t < NSC:
                    lhsT = dist_sb[:, (t // 4 % 4) * 400 + (t % 4) * 100:
                                   (t // 4 % 4) * 400 + (t % 4) * 100 + 100]
                else:
                    lhsT = sdist[:, (t - NSC) * 100:(t - NSC + 1) * 100]
                nc.tensor.matmul(
                    wt_ps[0:C, :], lhsT=lhsT,
                    rhs=zn[:, t * 256:(t + 1) * 256],
                    start=(t == 0), stop=(t == NT - 1),
                )

        gA = ctx.enter_context(tc.tile_pool(name="gA", bufs=1, space="PSUM"))
        gB = ctx.enter_context(tc.tile_pool(name="gB", bufs=1, space="PSUM"))
        gC = ctx.enter_context(tc.tile_pool(name="gC", bufs=1, space="PSUM"))
        gB2 = ctx.enter_context(tc.tile_pool(name="gB2", bufs=1, space="PSUM"))
        pw_cm = tc.tile_pool(name="pw", bufs=1, space="PSUM")
        pw = pw_cm.__enter__()
        wt_ps = pw.tile([128, 256], f32, tag="wt", name="wt_ps")

        def sims_into(g, rt, b, col0):
            for d in range(2):
                nc.tensor.matmul(
                    g[:, col0:col0 + 512],
                    lhsT=zT[:, d * SIMW + rt * 128: d * SIMW + (rt + 1) * 128],
                    rhs=zT[:, d * SIMW + b * 512: d * SIMW + (b + 1) * 512],
                    start=(d == 0), stop=(d == 1),
                )

        GOFF = {0: 0, 1: 1024, 2: 1536, 3: 2048}

        def exp_granule(g, rt, gi, width):
            nc.scalar.activation(
                out=exp_sb[:, rt * SIMW + GOFF[gi]:
                           rt * SIMW + GOFF[gi] + width],
                in_=g[:, 0:width], func=EXP,
                accum_out=out_sb[:, gi * 4 + rt: gi * 4 + rt + 1])

        def granules_for(rt):
            a = gA.tile([128, 1024], f32, tag="gA", name=f"g{rt}a")
            sims_into(a, rt, 0, 0)
            sims_into(a, rt, 1, 512)
            exp_granule(a, rt, 0, 1024)
            bb = gB.tile([128, 512], f32, tag="gB", name=f"g{rt}b")
            sims_into(bb, rt, 2, 0)
            exp_granule(bb, rt, 1, 512)
            b2 = gB2.tile([128, 512], f32, tag="gB2", name=f"g{rt}b2")
            sims_into(b2, rt, 3, 0)
            exp_granule(b2, rt, 2, 512)
            cc = gC.tile([128, 512], f32, tag="gC", name=f"g{rt}c")
            sims_into(cc, rt, 4, 0)
            exp_granule(cc, rt, 3, 512)

        # ---------------- chunks 0-4 + strip rt0 granules -------------------
        with tc.tile_pool(name="pt", bufs=2, space="PSUM") as ptp:
            a0 = b0 = None
            for ch in range(NBLK):
                tc.tile_set_cur_wait(arrival(ch) / 1000.0)
                norms_chunk(ch)
                tc.tile_set_cur_wait((arrival(ch) + 0.40) / 1000.0)
                rn_chain(4 * ch, 4, need_nn=True)
                tc.tile_set_cur_wait((arrival(ch) + 0.75) / 1000.0)
                scale_chunk(ch)
                tc.tile_set_cur_wait((arrival(ch) + 1.30) / 1000.0)
                transpose_chunk(ptp, ch)
                tc.tile_set_cur_wait((arrival(ch) + 1.95) / 1000.0)
                if ch == 0:
                    a0 = gA.tile([128, 1024], f32, tag="gA", name="g0a")
                    sims_into(a0, 0, 0, 0)
                elif ch == 1:
                    sims_into(a0, 0, 1, 512)
                    exp_granule(a0, 0, 0, 1024)
                elif ch == 2:
                    b0 = gB.tile([128, 512], f32, tag="gB", name="g0b")
                    sims_into(b0, 0, 2, 0)
                    exp_granule(b0, 0, 1, 512)
                elif ch == 3:
                    b02 = gB2.tile([128, 512], f32, tag="gB2", name="g0b2")
                    sims_into(b02, 0, 3, 0)
                    exp_granule(b02, 0, 2, 512)
                else:
                    c0 = gC.tile([128, 512], f32, tag="gC", name="g0c")
                    sims_into(c0, 0, 4, 0)
                    exp_granule(c0, 0, 3, 512)

        for ch in range(NBLK, NCH):
            tc.tile_set_cur_wait(arrival(ch) / 1000.0)
            norms_chunk(ch)
        tc.tile_set_cur_wait((arrival(7) + 0.40) / 1000.0)
        rn_chain(NSC, 12, need_nn=True)
        for t in range(NSC, NT):
            srcd = dist_sb[:, (t // 4 % 4) * 400 + (t % 4) * 100:
                           (t // 4 % 4) * 400 + (t % 4) * 100 + 100]
            nc.gpsimd.normalize_recip(
                out_ap=sdist[:, (t - NSC) * 100:(t - NSC + 1) * 100],
                in_ap=srcd.bitcast(f32),
                denom_ap=nn[:, t:t + 1])

        # ---------------- strips 1-3, W tail, P path ------------------------
        tc.tile_set_cur_wait(12.0 / 1000.0)
        w_tiles(range(0, NSC))
        granules_for(1)
        tc.tile_set_cur_wait((arrival(7) + 1.5) / 1000.0)
        w_tiles(range(NSC, NT))
        granules_for(2)

        nc.vector.tensor_copy(out=wt_sb[0:C, :], in_=wt_ps[0:C, :])
        for d in range(2):
            w_ps = pw.tile([128, 128], f32, tag="wt", name=f"w_ps{d}")
            nc.tensor.transpose(
                w_ps[:, 0:C],
                wt_sb[0:C, d * 128:(d + 1) * 128],
                ident[0:C, 0:C],
            )
            nc.vector.tensor_copy(out=w_sb[:, d * 256:d * 256 + C],
                                  in_=w_ps[:, 0:C])

        cc3 = gC.tile([128, 512], f32, tag="gC", name="g3c")
        sims_into(cc3, 3, 4, 0)
        exp_granule(cc3, 3, 3, 512)
        bb3 = gB.tile([128, 512], f32, tag="gB", name="g3b")
        sims_into(bb3, 3, 2, 0)
        exp_granule(bb3, 3, 1, 512)
        b32 = gB2.tile([128, 512], f32, tag="gB2", name="g3b2")
        sims_into(b32, 3, 3, 0)
        exp_granule(b32, 3, 2, 512)
        a3 = gA.tile([128, 1024], f32, tag="gA", name="g3a")
        sims_into(a3, 3, 0, 0)
        sims_into(a3, 3, 1, 512)
        exp_granule(a3, 3, 0, 1024)

        for rt in range(4):
            q_ps = pw.tile([128, 256], f32, tag="wt", name=f"q_ps{rt}")
            for d in range(2):
                nc.tensor.matmul(
                    q_ps[:], lhsT=zT[:, d * SIMW + rt * 128:
                                     d * SIMW + (rt + 1) * 128],
                    rhs=w_sb[:, d * 256:(d + 1) * 256],
                    start=(d == 0), stop=(d == 1),
                )
            nc.vector.scalar_tensor_tensor(
                out=junk[:], in0=q_ps[:, 0:C], scalar=1.0,
                in1=dist_sb[:, rt * 100:rt * 100 + 100].bitcast(f32),
                op0=MULT, op1=MULT,
                accum_out=out_sb[:, 16 + rt:17 + rt])
        pw_cm.__exit__(None, None, None)

        # ---------------- column sums of blocks B1-B3 -----------------------
        with tc.tile_pool(name="cs", bufs=1, space="PSUM") as csp:
            cs_ps = csp.tile([128, 1536], f32, tag="cs")
            for rt in range(4):
                for k in (1, 2, 3):
                    nc.tensor.matmul(
                        cs_ps[0:1, (k - 1) * 512:k * 512], lhsT=ones_r[:],
                        rhs=exp_sb[:, rt * SIMW + k * 512:
                                   rt * SIMW + (k + 1) * 512],
                        start=(rt == 0), stop=(rt == 3),
                    )
            nc.scalar.copy(out=cs_sb[:, 0:512], in_=cs_ps[0:1, 0:512])
            nc.vector.tensor_copy(out=cs_sb[:, 512:1024],
                                  in_=cs_ps[0:1, 512:1024])
            nc.scalar.copy(out=cs_sb[:, 1024:1536],
                           in_=cs_ps[0:1, 1024:1536])
            nc.sync.dma_start(out=out2[:], in_=cs_sb[:])

        nc.sync.dma_start(out=out[:], in_=out_sb[:])

    nc.finalize()
    return nc


def _get_program():
    global _PROG
    if _PROG is None:
        _PROG = _build_program()
    return _PROG


def kernel(z_i, z_j, z_n, dist_labels):
    from concourse.bass_utils import run_bass_kernel_spmd

    nc = _get_program()

    z_full = np.ascontiguousarray(
        np.concatenate([z_i, z_j], axis=0), dtype=np.float32
    )
    dist = np.ascontiguousarray(dist_labels, dtype=np.float32)

    in_maps = []
    for c in range(N_CORES):
        r0 = c * RPC
        in_maps.append({
            "z": np.ascontiguousarray(np.roll(z_full, -r0, axis=0)),
            "dist": np.ascontiguousarray(np.roll(dist, -r0, axis=0)),
        })

    res = run_bass_kernel_spmd(nc, in_maps, list(range(N_CORES))).results

    S = np.zeros(N, np.float64)
    P = np.empty(N, np.float64)
    idx = np.arange(RPC)
    for c in range(N_CORES):
        o = res[c]["out"].astype(np.float64)
        g = (idx + RPC * c) % N
        # device order p*4+rt == local row 4p+rt
        S[g] += (o[:, 0:4] + o[:, 4:8] + o[:, 8:12]
                 + o[:, 12:16]).reshape(RPC)
        P[g] = o[:, 16:20].reshape(RPC)
        cs = res[c]["out2"].astype(np.float64).reshape(3, 4, 128)
        for k in (1, 2, 3):
            # block col j=kk*128+p -> local row 512k + 4p + kk
            cs_r = cs[k - 1].T.reshape(RPC)
            gk = (idx + RPC * (c + k)) % N
            S[gk] += cs_r

    S -= E_CONST
    P -= 1.0
    return np.float32((P / S).sum() / N)
